# revision 1
# baseline (speedup 1.0000x reference)
"""Trainium2 Bass kernel for the 2-layer ReLU-RNN discriminator.

  B=64, T=512, I=256, H=512, O=1
  layer l: h_t = relu(x_t @ W_ih^T + b_ih + b_hh + h_{t-1} @ W_hh^T)
  out = sigmoid(h1 @ W_fc^T + b_fc)

Sharding: data-parallel over batch, 8 sequences per NeuronCore.

Per-core design (all activations/weights bf16, PSUM accumulation fp32):
- hidden state kept TRANSPOSED: column t*8+b of a [H=512(4x128 part-chunks), .]
  buffer holds h_t for local sample b. The recurrence then needs no
  transposes: stationary = W_hh^T tile [128,128], moving = h chunk [128,8],
  psum out [128(h_out chunk), 8] in the same transposed layout.
- x^T is prepared host-side (layout prep of the sharded input).
- xw = x @ W_ih^T + b precomputed as chunked GEMMs (64 timesteps/chunk).
- L0 and L1 recurrences are software-interleaved with a CH-step delay so the
  PE always has an independent matmul stream while the other layer's
  add+relu (both on DVE - keeping ACT out of the per-step chain measured ~3x
  faster) completes; otherwise the chain PE->DVE->PE stalls the PE each step.
- k-outer matmul order + per-m epilogues on 4 single-bank PSUM tiles let the
  next step's k=0 matmuls gate only on the m=0 epilogue.
"""

import numpy as np
import ml_dtypes

import concourse.bass as bass
import concourse.mybir as mybir
from concourse.tile import TileContext
from concourse.bass_utils import run_bass_kernel_spmd
from concourse.alu_op_type import AluOpType

BF16 = ml_dtypes.bfloat16
NCORES = 8
B, T, I, H, O = 64, 512, 256, 512, 1
BL = B // NCORES          # sequences per core
import os
CH = int(os.environ.get('K_CH', '32'))  # timesteps per chunk
NCH = T // CH             # chunks
KI = I // 128             # k-chunks of the input dim
KH = H // 128             # k-chunks / m-chunks of the hidden dim
W = BL * KH               # packed step width: 4 m-chunks x 8 samples = 32

_ctr = [0]


def _split_multi_waits(nc):
    """This container's walrus build rejects >1 sync-wait per instruction
    ("Too many sync wait commands"). Rewrite any instruction with N>1 waits
    into N-1 preceding single-wait NOPs on the same engine."""
    n_split = 0
    for f in nc.m.functions:
        for bb in f.blocks:
            out = []
            changed = False
            for inst in bb.instructions:
                si = inst.sync_info
                waits = list(si.on_wait) if si is not None and si.on_wait else []
                if len(waits) > 1:
                    changed = True
                    n_split += 1
                    for w in waits[:-1]:
                        _ctr[0] += 1
                        nop = mybir.InstNoOp(
                            name=f"waitnop-{_ctr[0]}", ins=[], outs=[]
                        )
                        nop.engine = inst.engine
                        nop.sync_info = mybir.SyncInfo(on_wait=[w], on_update=[])
                        out.append(nop)
                    inst.sync_info = mybir.SyncInfo(
                        on_wait=[waits[-1]],
                        on_update=list(si.on_update) if si.on_update else [],
                    )
                out.append(inst)
            if changed:
                bb.instructions = out
    return n_split


def build_nc(n_steps=T, split_waits=True, debug_dumps=False, delay=CH):
    nsc = n_steps // CH  # number of chunks actually used
    nc = bass.Bass("TRN2", num_devices=NCORES)
    f32, bf = mybir.dt.float32, mybir.dt.bfloat16

    xt_d = nc.dram_tensor("xt", [I, n_steps * BL], bf, kind="ExternalInput")
    w0i_d = nc.dram_tensor("w0i", [I, H], bf, kind="ExternalInput")
    w0h_d = nc.dram_tensor("w0h", [H, H], bf, kind="ExternalInput")
    w1i_d = nc.dram_tensor("w1i", [H, H], bf, kind="ExternalInput")
    w1h_d = nc.dram_tensor("w1h", [H, H], bf, kind="ExternalInput")
    MF = 8
    wfc_d = nc.dram_tensor("wfc", [128, KH * MF], bf, kind="ExternalInput")
    b0_d = nc.dram_tensor("b0", [128, KH], f32, kind="ExternalInput")
    b1_d = nc.dram_tensor("b1", [128, KH], f32, kind="ExternalInput")
    bfc_d = nc.dram_tensor("bfc", [1, 1], f32, kind="ExternalInput")
    y_d = nc.dram_tensor("y", [nsc, CH * BL], f32, kind="ExternalOutput")
    bf_np = mybir.dt.bfloat16
    if debug_dumps:
        dbg = {
            "dxw0": nc.dram_tensor("dxw0", [128, CH * W], bf_np, kind="ExternalOutput"),
            "dh0": nc.dram_tensor("dh0", [128, KH * CH * BL], bf_np, kind="ExternalOutput"),
            "dxw1": nc.dram_tensor("dxw1", [128, CH * W], bf_np, kind="ExternalOutput"),
            "dh1": nc.dram_tensor("dh1", [128, KH * CH * BL], bf_np, kind="ExternalOutput"),
        }

    with TileContext(nc) as tc:
        with (
            tc.tile_pool(name="xt", bufs=KI) as p_xt,
            tc.tile_pool(name="wts", bufs=6) as p_w,
            tc.tile_pool(name="h0", bufs=nsc) as p_h0,
            tc.tile_pool(name="h1", bufs=nsc) as p_h1,
            tc.tile_pool(name="xw0", bufs=nsc) as p_xw0,
            tc.tile_pool(name="xw1", bufs=nsc) as p_xw1,
            tc.tile_pool(name="z", bufs=16) as p_z,
            tc.tile_pool(name="fco", bufs=2) as p_fco,
            tc.tile_pool(name="psr", bufs=1, space="PSUM") as p_psr,
            tc.tile_pool(name="psg", bufs=2, space="PSUM") as p_psg,
            tc.tile_pool(name="psfc", bufs=2, space="PSUM") as p_psfc,
        ):
            # ---- load inputs to SBUF ----
            xt_sb = []
            for k in range(KI):
                t_ = p_xt.tile([128, n_steps * BL], bf, tag="xt", name=f"xtsb{k}")
                nc.sync.dma_start(t_[:], xt_d[k * 128:(k + 1) * 128, :])
                xt_sb.append(t_)

            def load_w(dram, kchunks):
                t_ = p_w.tile([128, kchunks * H], bf, tag="w", name=f"w{_ctr[0]}") ; _ctr[0] += 1
                for k in range(kchunks):
                    nc.sync.dma_start(
                        t_[:, k * H:(k + 1) * H], dram[k * 128:(k + 1) * 128, :]
                    )
                return t_

            w0i_sb = load_w(w0i_d, KI)
            w0h_sb = load_w(w0h_d, KH)
            w1i_sb = load_w(w1i_d, KH)
            w1h_sb = load_w(w1h_d, KH)
            wfc_sb = p_w.tile([128, KH * MF], bf, tag="small")
            nc.sync.dma_start(wfc_sb[:], wfc_d[:])
            b0_sb = p_w.tile([128, KH], f32, tag="small")
            nc.sync.dma_start(b0_sb[:], b0_d[:])
            b1_sb = p_w.tile([128, KH], f32, tag="small")
            nc.sync.dma_start(b1_sb[:], b1_d[:])
            bfc_sb = p_w.tile([1, 1], f32, tag="small")
            nc.sync.dma_start(bfc_sb[:], bfc_d[:])

            # persistent chunk tiles
            h0c = [p_h0.tile([128, KH * CH * BL], bf, tag="h0", name=f"h0c{i}")
                   for i in range(nsc)]
            h1c = [p_h1.tile([128, KH * CH * BL], bf, tag="h1", name=f"h1c{i}")
                   for i in range(nsc)]
            xw0c = [p_xw0.tile([128, CH * W], bf, tag="xw0", name=f"xw0c{i}")
                    for i in range(nsc)]
            xw1c = [p_xw1.tile([128, CH * W], bf, tag="xw1", name=f"xw1c{i}")
                    for i in range(nsc)]

            def r3_h(tile):   # [128, KH, CH*BL]
                return tile[:].rearrange("p (k x) -> p k x", k=KH)

            def r3_xw(tile):  # [128, CH, W]
                return tile[:].rearrange("p (t w) -> p t w", w=W)

            # ---- input GEMM for layer 0: xw0 = x @ W_ih0^T + b0 ----
            def gemm0(c):
                for m in range(KH):
                    ps = p_psg.tile([128, CH * BL], f32, tag="psg", name=f"psg{_ctr[0]}"); _ctr[0] += 1
                    for k in range(KI):
                        nc.tensor.matmul(
                            ps[:],
                            w0i_sb[:, k * H + m * 128: k * H + (m + 1) * 128],
                            xt_sb[k][:, c * CH * BL:(c + 1) * CH * BL],
                            start=(k == 0),
                            stop=(k == KI - 1),
                        )
                    nc.scalar.activation(
                        r3_xw(xw0c[c])[:, :, m * BL:(m + 1) * BL],
                        ps[:].rearrange("p (t b) -> p t b", b=BL),
                        mybir.ActivationFunctionType.Identity,
                        bias=b0_sb[:, m:m + 1],
                    )

            # ---- input GEMM for layer 1 (consumes finished h0 chunk) ----
            def gemm1(c):
                for m in range(KH):
                    ps = p_psg.tile([128, CH * BL], f32, tag="psg", name=f"psg{_ctr[0]}"); _ctr[0] += 1
                    for k in range(KH):
                        nc.tensor.matmul(
                            ps[:],
                            w1i_sb[:, k * H + m * 128: k * H + (m + 1) * 128],
                            h0c[c][:, k * CH * BL:(k + 1) * CH * BL],
                            start=(k == 0),
                            stop=(k == KH - 1),
                        )
                    nc.scalar.activation(
                        r3_xw(xw1c[c])[:, :, m * BL:(m + 1) * BL],
                        ps[:].rearrange("p (t b) -> p t b", b=BL),
                        mybir.ActivationFunctionType.Identity,
                        bias=b1_sb[:, m:m + 1],
                    )

            # ---- one recurrence step (shared by both layers) ----
            # 4 single-bank psum tiles, one per output m-chunk, shared by both
            # layers (their groups alternate in the PE stream). k-outer MM
            # order + per-m epilogue: relu for m completes while later k
            # groups still run, so the next step's k=0 matmuls (which need
            # only h chunk 0) are gated by an epilogue that finished early.
            def rec_step(t, whh_sb, xwc, hc, z_tag):
                c, r = divmod(t, CH)
                if t == 0:
                    # h_{-1} = 0: h_0 = relu(xw_0)
                    for m in range(KH):
                        nc.vector.tensor_scalar_max(
                            hc[0][:, m * CH * BL: m * CH * BL + BL],
                            xwc[0][:, m * BL:(m + 1) * BL], 0.0)
                    return
                pc, pr = divmod(t - 1, CH)
                ps0 = p_psr.tile([128, BL], f32, tag=z_tag + "p0",
                                 name=f"p0{z_tag}{t}")
                ps123 = p_psr.tile([128, (KH - 1) * BL], f32,
                                   tag=z_tag + "p123",
                                   name=f"p123{z_tag}{t}")

                def psl(m):
                    if m == 0:
                        return ps0[:]
                    return ps123[:, (m - 1) * BL:m * BL]

                for k in range(KH):
                    rhs = hc[pc][:, k * CH * BL + pr * BL:
                                 k * CH * BL + (pr + 1) * BL]
                    for m in range(KH):
                        nc.tensor.matmul(
                            psl(m),
                            whh_sb[:, k * H + m * 128: k * H + (m + 1) * 128],
                            rhs,
                            start=(k == 0),
                            stop=(k == KH - 1),
                        )
                        if k == KH - 1:
                            hdst = hc[c][:, m * CH * BL + r * BL:
                                         m * CH * BL + (r + 1) * BL]
                            xsl = xwc[c][:, r * W + m * BL:
                                         r * W + (m + 1) * BL]
                            if m == 0:
                                z = p_z.tile([128, BL], bf, tag=z_tag,
                                             name=f"z{z_tag}{t}m{m}")
                                nc.vector.tensor_tensor(
                                    z[:], ps0[:], xsl, AluOpType.add)
                                nc.vector.tensor_scalar_max(hdst, z[:], 0.0)
                            elif m == KH - 1:
                                z3 = p_z.tile([128, (KH - 1) * BL], bf,
                                              tag=z_tag + "w",
                                              name=f"zw{z_tag}{t}")
                                x3 = xwc[c][:, r * W + BL: r * W + KH * BL]
                                nc.vector.tensor_tensor(
                                    z3[:], ps123[:], x3, AluOpType.add)
                                nc.vector.tensor_scalar_max(
                                    r3_h(hc[c])[:, 1:KH,
                                                r * BL:(r + 1) * BL],
                                    z3[:].rearrange(
                                        "p (q b) -> p q b", b=BL),
                                    0.0)

            # ---- final FC + sigmoid for a finished h1 chunk ----
            def fc(c):
                ps = p_psfc.tile([MF, CH * BL], f32, tag="psfc", name=f"psfc{c}")
                for k in range(KH):
                    nc.tensor.matmul(
                        ps[:],
                        wfc_sb[:, k * MF:(k + 1) * MF],
                        h1c[c][:, k * CH * BL:(k + 1) * CH * BL],
                        start=(k == 0),
                        stop=(k == KH - 1),
                    )
                o = p_fco.tile([1, CH * BL], f32, tag="fco", name=f"fco{c}")
                nc.scalar.activation(
                    o[:], ps[0:1, :], mybir.ActivationFunctionType.Sigmoid,
                    bias=bfc_sb[0:1, 0:1],
                )
                nc.sync.dma_start(y_d[c:c + 1, :], o[:])

            # ---- interleaved schedule ----
            # gemm0 chunk 0 upfront; later chunks stream one ahead of the
            # L0 recurrence inside the loop so the PE head stays short.
            gemm0(0)
            if nsc > 1:
                gemm0(1)
            for t in range(n_steps + delay):
                if t < n_steps and t % CH == 0 and t // CH + 2 < nsc:
                    gemm0(t // CH + 2)
                if t < n_steps:
                    rec_step(t, w0h_sb, xw0c, h0c, "z0")
                if t >= delay:
                    _xw1 = xw0c if os.environ.get("K_NO_GEMM1") else xw1c
                    rec_step(t - delay, w1h_sb, _xw1, h1c, "z1")
                if t < n_steps and (t + 1) % CH == 0:
                    if not os.environ.get("K_NO_GEMM1"):
                        gemm1((t + 1) // CH - 1)
                if t >= delay and (t - delay + 1) % CH == 0:
                    if not os.environ.get("K_NO_FC"):
                        fc((t - delay + 1) // CH - 1)
            if os.environ.get("K_NO_FC"):
                # still need the y output written once
                o0 = p_fco.tile([1, CH * BL], f32, tag="fco", name="fco_x")
                nc.vector.memset(o0[:], 0.0)
                for c in range(nsc):
                    nc.sync.dma_start(y_d[c:c + 1, :], o0[:])
            if debug_dumps:
                dc = int(os.environ.get("K_DBG_CHUNK", "0"))
                nc.sync.dma_start(dbg["dxw0"][:], xw0c[dc][:])
                nc.sync.dma_start(dbg["dh0"][:], h0c[dc][:])
                nc.sync.dma_start(dbg["dxw1"][:], xw1c[dc][:])
                nc.sync.dma_start(dbg["dh1"][:], h1c[dc][:])

    if split_waits:
        _split_multi_waits(nc)
    return nc


_cache = {}


def _get_nc(n_steps):
    if n_steps not in _cache:
        _cache[n_steps] = build_nc(n_steps)
    return _cache[n_steps]


def _wfc_host(W_fc):
    MF = 8
    w = np.zeros((KH, 128, MF), np.float32)
    w[:, :, 0] = W_fc.reshape(KH, 128)
    return np.ascontiguousarray(w.transpose(1, 0, 2).reshape(128, KH * MF)).astype(BF16)


def _prep_inputs(x, W_ih0, W_hh0, b_ih0, b_hh0, W_ih1, W_hh1, b_ih1, b_hh1,
                 W_fc, b_fc, n_steps=T):
    shared = {
        "w0i": np.ascontiguousarray(W_ih0.T).astype(BF16),
        "w0h": np.ascontiguousarray(W_hh0.T).astype(BF16),
        "w1i": np.ascontiguousarray(W_ih1.T).astype(BF16),
        "w1h": np.ascontiguousarray(W_hh1.T).astype(BF16),
        "wfc": _wfc_host(W_fc),
        "b0": np.ascontiguousarray((b_ih0 + b_hh0).reshape(KH, 128).T).astype(
            np.float32),
        "b1": np.ascontiguousarray((b_ih1 + b_hh1).reshape(KH, 128).T).astype(
            np.float32),
        "bfc": b_fc.reshape(1, 1).astype(np.float32),
    }
    in_maps = []
    for c in range(NCORES):
        xs = x[c * BL:(c + 1) * BL, :n_steps]          # [BL, n_steps, I]
        xt = np.ascontiguousarray(xs.transpose(2, 1, 0)).reshape(
            I, n_steps * BL)                            # col = t*BL + b
        in_maps.append({"xt": xt.astype(BF16), **shared})
    return in_maps


def _postprocess(results, n_steps=T):
    outs = []
    for c in range(NCORES):
        y = results[c]["y"].reshape(n_steps, BL)        # [t, b]
        outs.append(y.T)                                # [b, t]
    return np.concatenate(outs, axis=0)[:, :, None].astype(np.float32)


def kernel(x, W_ih0, W_hh0, b_ih0, b_hh0, W_ih1, W_hh1, b_ih1, b_hh1,
           W_fc, b_fc):
    x, W_ih0, W_hh0, b_ih0, b_hh0, W_ih1, W_hh1, b_ih1, b_hh1, W_fc, b_fc = [
        np.asarray(a, dtype=np.float32)
        for a in (x, W_ih0, W_hh0, b_ih0, b_hh0, W_ih1, W_hh1, b_ih1, b_hh1,
                  W_fc, b_fc)
    ]
    nc = _get_nc(T)
    in_maps = _prep_inputs(x, W_ih0, W_hh0, b_ih0, b_hh0, W_ih1, W_hh1,
                           b_ih1, b_hh1, W_fc, b_fc)
    res = run_bass_kernel_spmd(nc, in_maps, core_ids=list(range(NCORES)))
    return _postprocess(res.results)



# revision 3
# speedup vs baseline: 1.0110x; 1.0110x over previous
"""Trainium2 Bass kernel for the 2-layer ReLU-RNN discriminator.

  B=64, T=512, I=256, H=512, O=1
  layer l: h_t = relu(x_t @ W_ih^T + b_ih + b_hh + h_{t-1} @ W_hh^T)
  out = sigmoid(h1 @ W_fc^T + b_fc)

Sharding: data-parallel over batch, 8 sequences per NeuronCore.

Per-core design (all activations/weights bf16, PSUM accumulation fp32):
- hidden state kept TRANSPOSED: column t*8+b of a [H=512(4x128 part-chunks), .]
  buffer holds h_t for local sample b. The recurrence then needs no
  transposes: stationary = W_hh^T tile [128,128], moving = h chunk [128,8],
  psum out [128(h_out chunk), 8] in the same transposed layout.
- x^T is prepared host-side (layout prep of the sharded input).
- xw = x @ W_ih^T + b precomputed as chunked GEMMs (64 timesteps/chunk).
- L0 and L1 recurrences are software-interleaved with a CH-step delay so the
  PE always has an independent matmul stream while the other layer's
  add+relu (both on DVE - keeping ACT out of the per-step chain measured ~3x
  faster) completes; otherwise the chain PE->DVE->PE stalls the PE each step.
- k-outer matmul order + per-m epilogues on 4 single-bank PSUM tiles let the
  next step's k=0 matmuls gate only on the m=0 epilogue.
"""

import numpy as np
import ml_dtypes

import concourse.bass as bass
import concourse.mybir as mybir
from concourse.tile import TileContext
from concourse.bass_utils import run_bass_kernel_spmd
from concourse.alu_op_type import AluOpType

BF16 = ml_dtypes.bfloat16
NCORES = 8
B, T, I, H, O = 64, 512, 256, 512, 1
BL = B // NCORES          # sequences per core
import os
CH = int(os.environ.get('K_CH', '32'))  # timesteps per chunk
NCH = T // CH             # chunks
KI = I // 128             # k-chunks of the input dim
KH = H // 128             # k-chunks / m-chunks of the hidden dim
W = BL * KH               # packed step width: 4 m-chunks x 8 samples = 32

_ctr = [0]


def _split_multi_waits(nc):
    """This container's walrus build rejects >1 sync-wait per instruction
    ("Too many sync wait commands"). Rewrite any instruction with N>1 waits
    into N-1 preceding single-wait NOPs on the same engine."""
    n_split = 0
    for f in nc.m.functions:
        for bb in f.blocks:
            out = []
            changed = False
            for inst in bb.instructions:
                si = inst.sync_info
                waits = list(si.on_wait) if si is not None and si.on_wait else []
                if len(waits) > 1:
                    changed = True
                    n_split += 1
                    for w in waits[:-1]:
                        _ctr[0] += 1
                        nop = mybir.InstNoOp(
                            name=f"waitnop-{_ctr[0]}", ins=[], outs=[]
                        )
                        nop.engine = inst.engine
                        nop.sync_info = mybir.SyncInfo(on_wait=[w], on_update=[])
                        out.append(nop)
                    inst.sync_info = mybir.SyncInfo(
                        on_wait=[waits[-1]],
                        on_update=list(si.on_update) if si.on_update else [],
                    )
                out.append(inst)
            if changed:
                bb.instructions = out
    return n_split


def build_nc(n_steps=T, split_waits=True, debug_dumps=False, delay=CH):
    nsc = n_steps // CH  # number of chunks actually used
    nc = bass.Bass("TRN2", num_devices=NCORES)
    f32, bf = mybir.dt.float32, mybir.dt.bfloat16

    xt_d = nc.dram_tensor("xt", [I, n_steps * BL], bf, kind="ExternalInput")
    w0i_d = nc.dram_tensor("w0i", [I, H], bf, kind="ExternalInput")
    w0h_d = nc.dram_tensor("w0h", [H, H], bf, kind="ExternalInput")
    w1i_d = nc.dram_tensor("w1i", [H, H], bf, kind="ExternalInput")
    w1h_d = nc.dram_tensor("w1h", [H, H], bf, kind="ExternalInput")
    MF = 8
    wfc_d = nc.dram_tensor("wfc", [128, KH * MF], bf, kind="ExternalInput")
    b0_d = nc.dram_tensor("b0", [128, KH], f32, kind="ExternalInput")
    b1_d = nc.dram_tensor("b1", [128, KH], f32, kind="ExternalInput")
    bfc_d = nc.dram_tensor("bfc", [1, 1], f32, kind="ExternalInput")
    y_d = nc.dram_tensor("y", [nsc, CH * BL], f32, kind="ExternalOutput")
    bf_np = mybir.dt.bfloat16
    if debug_dumps:
        dbg = {
            "dxw0": nc.dram_tensor("dxw0", [128, CH * W], bf_np, kind="ExternalOutput"),
            "dh0": nc.dram_tensor("dh0", [128, KH * CH * BL], bf_np, kind="ExternalOutput"),
            "dxw1": nc.dram_tensor("dxw1", [128, CH * W], bf_np, kind="ExternalOutput"),
            "dh1": nc.dram_tensor("dh1", [128, KH * CH * BL], bf_np, kind="ExternalOutput"),
        }

    with TileContext(nc) as tc:
        with (
            tc.tile_pool(name="xt", bufs=KI) as p_xt,
            tc.tile_pool(name="wts", bufs=6) as p_w,
            tc.tile_pool(name="h0", bufs=nsc) as p_h0,
            tc.tile_pool(name="h1", bufs=nsc) as p_h1,
            tc.tile_pool(name="xw0", bufs=nsc) as p_xw0,
            tc.tile_pool(name="xw1", bufs=nsc) as p_xw1,
            tc.tile_pool(name="z", bufs=16) as p_z,
            tc.tile_pool(name="fco", bufs=2) as p_fco,
            tc.tile_pool(name="psr", bufs=2, space="PSUM") as p_psr,
            tc.tile_pool(name="psg", bufs=2, space="PSUM") as p_psg,
            tc.tile_pool(name="psfc", bufs=2, space="PSUM") as p_psfc,
        ):
            # ---- load inputs to SBUF ----
            xt_sb = []
            for k in range(KI):
                t_ = p_xt.tile([128, n_steps * BL], bf, tag="xt", name=f"xtsb{k}")
                nc.sync.dma_start(t_[:], xt_d[k * 128:(k + 1) * 128, :])
                xt_sb.append(t_)

            def load_w(dram, kchunks):
                t_ = p_w.tile([128, kchunks * H], bf, tag="w", name=f"w{_ctr[0]}") ; _ctr[0] += 1
                for k in range(kchunks):
                    nc.sync.dma_start(
                        t_[:, k * H:(k + 1) * H], dram[k * 128:(k + 1) * 128, :]
                    )
                return t_

            w0i_sb = load_w(w0i_d, KI)
            w0h_sb = load_w(w0h_d, KH)
            w1i_sb = load_w(w1i_d, KH)
            w1h_sb = load_w(w1h_d, KH)
            wfc_sb = p_w.tile([128, KH * MF], bf, tag="small")
            nc.sync.dma_start(wfc_sb[:], wfc_d[:])
            b0_sb = p_w.tile([128, KH], f32, tag="small")
            nc.sync.dma_start(b0_sb[:], b0_d[:])
            b1_sb = p_w.tile([128, KH], f32, tag="small")
            nc.sync.dma_start(b1_sb[:], b1_d[:])
            bfc_sb = p_w.tile([1, 1], f32, tag="small")
            nc.sync.dma_start(bfc_sb[:], bfc_d[:])

            # persistent chunk tiles
            h0c = [p_h0.tile([128, KH * CH * BL], bf, tag="h0", name=f"h0c{i}")
                   for i in range(nsc)]
            h1c = [p_h1.tile([128, KH * CH * BL], bf, tag="h1", name=f"h1c{i}")
                   for i in range(nsc)]
            xw0c = [p_xw0.tile([128, CH * W], bf, tag="xw0", name=f"xw0c{i}")
                    for i in range(nsc)]
            xw1c = [p_xw1.tile([128, CH * W], bf, tag="xw1", name=f"xw1c{i}")
                    for i in range(nsc)]

            def r3_h(tile):   # [128, KH, CH*BL]
                return tile[:].rearrange("p (k x) -> p k x", k=KH)

            def r3_xw(tile):  # [128, CH, W]
                return tile[:].rearrange("p (t w) -> p t w", w=W)

            # ---- input GEMM for layer 0: xw0 = x @ W_ih0^T + b0 ----
            def gemm0(c):
                for m in range(KH):
                    ps = p_psg.tile([128, CH * BL], f32, tag="psg", name=f"psg{_ctr[0]}"); _ctr[0] += 1
                    for k in range(KI):
                        nc.tensor.matmul(
                            ps[:],
                            w0i_sb[:, k * H + m * 128: k * H + (m + 1) * 128],
                            xt_sb[k][:, c * CH * BL:(c + 1) * CH * BL],
                            start=(k == 0),
                            stop=(k == KI - 1),
                        )
                    nc.scalar.activation(
                        r3_xw(xw0c[c])[:, :, m * BL:(m + 1) * BL],
                        ps[:].rearrange("p (t b) -> p t b", b=BL),
                        mybir.ActivationFunctionType.Identity,
                        bias=b0_sb[:, m:m + 1],
                    )

            # ---- input GEMM for layer 1 (consumes finished h0 chunk) ----
            def gemm1(c):
                for m in range(KH):
                    ps = p_psg.tile([128, CH * BL], f32, tag="psg", name=f"psg{_ctr[0]}"); _ctr[0] += 1
                    for k in range(KH):
                        nc.tensor.matmul(
                            ps[:],
                            w1i_sb[:, k * H + m * 128: k * H + (m + 1) * 128],
                            h0c[c][:, k * CH * BL:(k + 1) * CH * BL],
                            start=(k == 0),
                            stop=(k == KH - 1),
                        )
                    nc.scalar.activation(
                        r3_xw(xw1c[c])[:, :, m * BL:(m + 1) * BL],
                        ps[:].rearrange("p (t b) -> p t b", b=BL),
                        mybir.ActivationFunctionType.Identity,
                        bias=b1_sb[:, m:m + 1],
                    )

            # ---- one recurrence step (shared by both layers) ----
            # One [128, 32] psum tile (single bank) per layer per step,
            # double-buffered (bufs=2): step t uses buffer t%2, so its
            # start=True matmuls only WAR-depend on step t-2's epilogue read
            # (long done) instead of t-1's — the PE never waits on the DVE.
            # Single fused epilogue per step: TT add (+xw) then relu.
            def rec_step(t, whh_sb, xwc, hc, z_tag):
                c, r = divmod(t, CH)
                if t == 0:
                    # h_{-1} = 0: h_0 = relu(xw_0)
                    for m in range(KH):
                        nc.vector.tensor_scalar_max(
                            hc[0][:, m * CH * BL: m * CH * BL + BL],
                            xwc[0][:, m * BL:(m + 1) * BL], 0.0)
                    return
                pc, pr = divmod(t - 1, CH)
                ps = p_psr.tile([128, KH * BL], f32, tag=z_tag + "ps",
                                name=f"ps{z_tag}{t}")
                for k in range(KH):
                    rhs = hc[pc][:, k * CH * BL + pr * BL:
                                 k * CH * BL + (pr + 1) * BL]
                    for m in range(KH):
                        nc.tensor.matmul(
                            ps[:, m * BL:(m + 1) * BL],
                            whh_sb[:, k * H + m * 128: k * H + (m + 1) * 128],
                            rhs,
                            start=(k == 0),
                            stop=(k == KH - 1),
                        )
                z = p_z.tile([128, KH * BL], bf, tag=z_tag,
                             name=f"z{z_tag}{t}")
                nc.vector.tensor_tensor(
                    z[:], ps[:], xwc[c][:, r * W:(r + 1) * W], AluOpType.add)
                nc.vector.tensor_scalar_max(
                    r3_h(hc[c])[:, :, r * BL:(r + 1) * BL],
                    z[:].rearrange("p (q b) -> p q b", b=BL),
                    0.0)

            # ---- final FC + sigmoid for a finished h1 chunk ----
            def fc(c):
                ps = p_psfc.tile([MF, CH * BL], f32, tag="psfc", name=f"psfc{c}")
                for k in range(KH):
                    nc.tensor.matmul(
                        ps[:],
                        wfc_sb[:, k * MF:(k + 1) * MF],
                        h1c[c][:, k * CH * BL:(k + 1) * CH * BL],
                        start=(k == 0),
                        stop=(k == KH - 1),
                    )
                o = p_fco.tile([1, CH * BL], f32, tag="fco", name=f"fco{c}")
                nc.scalar.activation(
                    o[:], ps[0:1, :], mybir.ActivationFunctionType.Sigmoid,
                    bias=bfc_sb[0:1, 0:1],
                )
                nc.sync.dma_start(y_d[c:c + 1, :], o[:])

            # ---- interleaved schedule ----
            # gemm0 chunk 0 upfront; later chunks stream one ahead of the
            # L0 recurrence inside the loop so the PE head stays short.
            gemm0(0)
            if nsc > 1:
                gemm0(1)
            for t in range(n_steps + delay):
                if t < n_steps and t % CH == 0 and t // CH + 2 < nsc:
                    gemm0(t // CH + 2)
                if t < n_steps:
                    rec_step(t, w0h_sb, xw0c, h0c, "z0")
                if t >= delay:
                    _xw1 = xw0c if os.environ.get("K_NO_GEMM1") else xw1c
                    rec_step(t - delay, w1h_sb, _xw1, h1c, "z1")
                if t < n_steps and (t + 1) % CH == 0:
                    if not os.environ.get("K_NO_GEMM1"):
                        gemm1((t + 1) // CH - 1)
                if t >= delay and (t - delay + 1) % CH == 0:
                    if not os.environ.get("K_NO_FC"):
                        fc((t - delay + 1) // CH - 1)
            if os.environ.get("K_NO_FC"):
                # still need the y output written once
                o0 = p_fco.tile([1, CH * BL], f32, tag="fco", name="fco_x")
                nc.vector.memset(o0[:], 0.0)
                for c in range(nsc):
                    nc.sync.dma_start(y_d[c:c + 1, :], o0[:])
            if debug_dumps:
                dc = int(os.environ.get("K_DBG_CHUNK", "0"))
                nc.sync.dma_start(dbg["dxw0"][:], xw0c[dc][:])
                nc.sync.dma_start(dbg["dh0"][:], h0c[dc][:])
                nc.sync.dma_start(dbg["dxw1"][:], xw1c[dc][:])
                nc.sync.dma_start(dbg["dh1"][:], h1c[dc][:])

    if split_waits:
        _split_multi_waits(nc)
    return nc


_cache = {}


def _get_nc(n_steps):
    if n_steps not in _cache:
        _cache[n_steps] = build_nc(n_steps)
    return _cache[n_steps]


def _wfc_host(W_fc):
    MF = 8
    w = np.zeros((KH, 128, MF), np.float32)
    w[:, :, 0] = W_fc.reshape(KH, 128)
    return np.ascontiguousarray(w.transpose(1, 0, 2).reshape(128, KH * MF)).astype(BF16)


def _prep_inputs(x, W_ih0, W_hh0, b_ih0, b_hh0, W_ih1, W_hh1, b_ih1, b_hh1,
                 W_fc, b_fc, n_steps=T):
    shared = {
        "w0i": np.ascontiguousarray(W_ih0.T).astype(BF16),
        "w0h": np.ascontiguousarray(W_hh0.T).astype(BF16),
        "w1i": np.ascontiguousarray(W_ih1.T).astype(BF16),
        "w1h": np.ascontiguousarray(W_hh1.T).astype(BF16),
        "wfc": _wfc_host(W_fc),
        "b0": np.ascontiguousarray((b_ih0 + b_hh0).reshape(KH, 128).T).astype(
            np.float32),
        "b1": np.ascontiguousarray((b_ih1 + b_hh1).reshape(KH, 128).T).astype(
            np.float32),
        "bfc": b_fc.reshape(1, 1).astype(np.float32),
    }
    in_maps = []
    for c in range(NCORES):
        xs = x[c * BL:(c + 1) * BL, :n_steps]          # [BL, n_steps, I]
        xt = np.ascontiguousarray(xs.transpose(2, 1, 0)).reshape(
            I, n_steps * BL)                            # col = t*BL + b
        in_maps.append({"xt": xt.astype(BF16), **shared})
    return in_maps


def _postprocess(results, n_steps=T):
    outs = []
    for c in range(NCORES):
        y = results[c]["y"].reshape(n_steps, BL)        # [t, b]
        outs.append(y.T)                                # [b, t]
    return np.concatenate(outs, axis=0)[:, :, None].astype(np.float32)


def kernel(x, W_ih0, W_hh0, b_ih0, b_hh0, W_ih1, W_hh1, b_ih1, b_hh1,
           W_fc, b_fc):
    x, W_ih0, W_hh0, b_ih0, b_hh0, W_ih1, W_hh1, b_ih1, b_hh1, W_fc, b_fc = [
        np.asarray(a, dtype=np.float32)
        for a in (x, W_ih0, W_hh0, b_ih0, b_hh0, W_ih1, W_hh1, b_ih1, b_hh1,
                  W_fc, b_fc)
    ]
    nc = _get_nc(T)
    in_maps = _prep_inputs(x, W_ih0, W_hh0, b_ih0, b_hh0, W_ih1, W_hh1,
                           b_ih1, b_hh1, W_fc, b_fc)
    res = run_bass_kernel_spmd(nc, in_maps, core_ids=list(range(NCORES)))
    return _postprocess(res.results)



# revision 11
# speedup vs baseline: 1.1542x; 1.1416x over previous
"""Trainium2 Bass kernel for the 2-layer ReLU-RNN discriminator.

  B=64, T=512, I=256, H=512, O=1
  layer l: h_t = relu(x_t @ W_ih^T + b_ih + b_hh + h_{t-1} @ W_hh^T)
  out = sigmoid(h1 @ W_fc^T + b_fc)

Sharding: data-parallel over batch, 8 sequences per NeuronCore.

Per-core design (all activations/weights bf16, PSUM accumulation fp32):
- hidden state kept TRANSPOSED: column t*8+b of a [H=512(4x128 part-chunks), .]
  buffer holds h_t for local sample b. The recurrence then needs no
  transposes: stationary = W_hh^T tile [128,128], moving = h chunk [128,8],
  psum out [128(h_out chunk), 8] in the same transposed layout.
- x^T is prepared host-side (layout prep of the sharded input).
- xw = x @ W_ih^T + b precomputed as chunked GEMMs (64 timesteps/chunk).
- L0 and L1 recurrences are software-interleaved with a CH-step delay so the
  PE always has an independent matmul stream while the other layer's
  add+relu (both on DVE - keeping ACT out of the per-step chain measured ~3x
  faster) completes; otherwise the chain PE->DVE->PE stalls the PE each step.
- one [128,32] single-bank PSUM tile per layer per step, double-buffered
  (pool bufs=2), so a step's start=True matmuls WAR-depend only on the
  epilogue two steps back; single fused TT-add + relu epilogue per step.
- K_COMPACT: the recurrence matmuls read h from small rotating [128,32]
  tiles (written by an extra per-step relu) instead of strided slices of the
  wide per-chunk archive tiles; wide-AP moving-operand reads measured ~+40%
  per-MM on HW.
"""

import numpy as np
import ml_dtypes

import concourse.bass as bass
import concourse.mybir as mybir
from concourse.tile import TileContext
from concourse.bass_utils import run_bass_kernel_spmd
from concourse.alu_op_type import AluOpType

BF16 = ml_dtypes.bfloat16
NCORES = 8
B, T, I, H, O = 64, 512, 256, 512, 1
BL = B // NCORES          # sequences per core
import os
CH = int(os.environ.get('K_CH', '32'))  # timesteps per chunk
NCH = T // CH             # chunks
KI = I // 128             # k-chunks of the input dim
KH = H // 128             # k-chunks / m-chunks of the hidden dim
W = BL * KH               # packed step width: 4 m-chunks x 8 samples = 32

_ctr = [0]


def _split_multi_waits(nc):
    """This container's walrus build rejects >1 sync-wait per instruction
    ("Too many sync wait commands"). Rewrite any instruction with N>1 waits
    into N-1 preceding single-wait NOPs on the same engine."""
    n_split = 0
    for f in nc.m.functions:
        for bb in f.blocks:
            out = []
            changed = False
            for inst in bb.instructions:
                si = inst.sync_info
                waits = list(si.on_wait) if si is not None and si.on_wait else []
                if len(waits) > 1:
                    changed = True
                    n_split += 1
                    for w in waits[:-1]:
                        _ctr[0] += 1
                        nop = mybir.InstNoOp(
                            name=f"waitnop-{_ctr[0]}", ins=[], outs=[]
                        )
                        nop.engine = inst.engine
                        nop.sync_info = mybir.SyncInfo(on_wait=[w], on_update=[])
                        out.append(nop)
                    inst.sync_info = mybir.SyncInfo(
                        on_wait=[waits[-1]],
                        on_update=list(si.on_update) if si.on_update else [],
                    )
                out.append(inst)
            if changed:
                bb.instructions = out
    return n_split


def build_nc(n_steps=T, split_waits=True, debug_dumps=False, delay=CH):
    nsc = n_steps // CH  # number of chunks actually used
    nc = bass.Bass("TRN2", num_devices=NCORES)
    f32, bf = mybir.dt.float32, mybir.dt.bfloat16

    xt_d = nc.dram_tensor("xt", [I, n_steps * BL], bf, kind="ExternalInput")
    w0i_d = nc.dram_tensor("w0i", [I, H], bf, kind="ExternalInput")
    w0h_d = nc.dram_tensor("w0h", [H, H], bf, kind="ExternalInput")
    w1i_d = nc.dram_tensor("w1i", [H, H], bf, kind="ExternalInput")
    w1h_d = nc.dram_tensor("w1h", [H, H], bf, kind="ExternalInput")
    MF = 8
    wfc_d = nc.dram_tensor("wfc", [128, KH * MF], bf, kind="ExternalInput")
    b0_d = nc.dram_tensor("b0", [128, KH], f32, kind="ExternalInput")
    b1_d = nc.dram_tensor("b1", [128, KH], f32, kind="ExternalInput")
    bfc_d = nc.dram_tensor("bfc", [1, 1], f32, kind="ExternalInput")
    y_d = nc.dram_tensor("y", [nsc, CH * BL], f32, kind="ExternalOutput")
    bf_np = mybir.dt.bfloat16
    if debug_dumps:
        dbg = {
            "dxw0": nc.dram_tensor("dxw0", [128, CH * W], bf_np, kind="ExternalOutput"),
            "dh0": nc.dram_tensor("dh0", [128, KH * CH * BL], bf_np, kind="ExternalOutput"),
            "dxw1": nc.dram_tensor("dxw1", [128, CH * W], bf_np, kind="ExternalOutput"),
            "dh1": nc.dram_tensor("dh1", [128, KH * CH * BL], bf_np, kind="ExternalOutput"),
        }

    with TileContext(nc) as tc:
        with (
            tc.tile_pool(name="xt", bufs=KI) as p_xt,
            tc.tile_pool(name="wts", bufs=6) as p_w,
            tc.tile_pool(name="h0", bufs=nsc) as p_h0,
            tc.tile_pool(name="h1", bufs=nsc) as p_h1,
            tc.tile_pool(name="xw0", bufs=nsc) as p_xw0,
            tc.tile_pool(name="xw1", bufs=nsc) as p_xw1,
            tc.tile_pool(name="z", bufs=16) as p_z,
            tc.tile_pool(name="fco", bufs=2) as p_fco,
            tc.tile_pool(name="psr", bufs=2, space="PSUM") as p_psr,
            tc.tile_pool(name="psg", bufs=2, space="PSUM") as p_psg,
            tc.tile_pool(name="psfc", bufs=2, space="PSUM") as p_psfc,
        ):
            # ---- load inputs to SBUF ----
            xt_sb = []
            for k in range(KI):
                t_ = p_xt.tile([128, n_steps * BL], bf, tag="xt", name=f"xtsb{k}")
                nc.sync.dma_start(t_[:], xt_d[k * 128:(k + 1) * 128, :])
                xt_sb.append(t_)

            def load_w(dram, kchunks):
                t_ = p_w.tile([128, kchunks * H], bf, tag="w", name=f"w{_ctr[0]}") ; _ctr[0] += 1
                for k in range(kchunks):
                    nc.sync.dma_start(
                        t_[:, k * H:(k + 1) * H], dram[k * 128:(k + 1) * 128, :]
                    )
                return t_

            w0i_sb = load_w(w0i_d, KI)
            w0h_sb = load_w(w0h_d, KH)
            w1i_sb = load_w(w1i_d, KH)
            w1h_sb = load_w(w1h_d, KH)
            wfc_sb = p_w.tile([128, KH * MF], bf, tag="small")
            nc.sync.dma_start(wfc_sb[:], wfc_d[:])
            b0_sb = p_w.tile([128, KH], f32, tag="small")
            nc.sync.dma_start(b0_sb[:], b0_d[:])
            b1_sb = p_w.tile([128, KH], f32, tag="small")
            nc.sync.dma_start(b1_sb[:], b1_d[:])
            bfc_sb = p_w.tile([1, 1], f32, tag="small")
            nc.sync.dma_start(bfc_sb[:], bfc_d[:])

            # persistent chunk tiles
            h0c = [p_h0.tile([128, KH * CH * BL], bf, tag="h0", name=f"h0c{i}")
                   for i in range(nsc)]
            h1c = [p_h1.tile([128, KH * CH * BL], bf, tag="h1", name=f"h1c{i}")
                   for i in range(nsc)]
            if os.environ.get('K_EPI', 'real') != 'real':
                for tl in h0c + h1c:
                    nc.vector.memset(tl[:], 0.01)

            # compact per-step h tiles for the recurrence reads (the wide
            # archive tiles stay for gemm1/fc); rotating, one per layer
            K_COMPACT = bool(os.environ.get('K_COMPACT'))
            NHB = 3
            hcur = {
                "z0": [p_z.tile([128, KH * BL], bf, tag="hc0", name=f"hc0_{i}")
                       for i in range(NHB)],
                "z1": [p_z.tile([128, KH * BL], bf, tag="hc1", name=f"hc1_{i}")
                       for i in range(NHB)],
            } if K_COMPACT else None
            xw0c = [p_xw0.tile([128, CH * W], bf, tag="xw0", name=f"xw0c{i}")
                    for i in range(nsc)]
            xw1c = [p_xw1.tile([128, CH * W], bf, tag="xw1", name=f"xw1c{i}")
                    for i in range(nsc)]

            def r3_h(tile):   # [128, KH, CH*BL]
                return tile[:].rearrange("p (k x) -> p k x", k=KH)

            def r3_xw(tile):  # [128, CH, W]
                return tile[:].rearrange("p (t w) -> p t w", w=W)

            # ---- input GEMM for layer 0: xw0 = x @ W_ih0^T + b0 ----
            def gemm0(c):
                for m in range(KH):
                    ps = p_psg.tile([128, CH * BL], f32, tag="psg", name=f"psg{_ctr[0]}"); _ctr[0] += 1
                    for k in range(KI):
                        nc.tensor.matmul(
                            ps[:],
                            w0i_sb[:, k * H + m * 128: k * H + (m + 1) * 128],
                            xt_sb[k][:, c * CH * BL:(c + 1) * CH * BL],
                            start=(k == 0),
                            stop=(k == KI - 1),
                        )
                    nc.scalar.activation(
                        r3_xw(xw0c[c])[:, :, m * BL:(m + 1) * BL],
                        ps[:].rearrange("p (t b) -> p t b", b=BL),
                        mybir.ActivationFunctionType.Identity,
                        bias=b0_sb[:, m:m + 1],
                    )

            # ---- input GEMM for layer 1 (consumes finished h0 chunk) ----
            def gemm1(c):
                for m in range(KH):
                    ps = p_psg.tile([128, CH * BL], f32, tag="psg", name=f"psg{_ctr[0]}"); _ctr[0] += 1
                    for k in range(KH):
                        nc.tensor.matmul(
                            ps[:],
                            w1i_sb[:, k * H + m * 128: k * H + (m + 1) * 128],
                            h0c[c][:, k * CH * BL:(k + 1) * CH * BL],
                            start=(k == 0),
                            stop=(k == KH - 1),
                        )
                    nc.scalar.activation(
                        r3_xw(xw1c[c])[:, :, m * BL:(m + 1) * BL],
                        ps[:].rearrange("p (t b) -> p t b", b=BL),
                        mybir.ActivationFunctionType.Identity,
                        bias=b1_sb[:, m:m + 1],
                    )

            # ---- one recurrence step (shared by both layers) ----
            # One [128, 32] psum tile (single bank) per layer per step,
            # double-buffered (bufs=2): step t uses buffer t%2, so its
            # start=True matmuls only WAR-depend on step t-2's epilogue read
            # (long done) instead of t-1's — the PE never waits on the DVE.
            # Single fused epilogue per step: TT add (+xw) then relu.
            K_EPI = os.environ.get('K_EPI', 'real')

            def rec_step(t, whh_sb, xwc, hc, z_tag):
                c, r = divmod(t, CH)
                if t == 0:
                    if K_EPI == 'none':
                        return
                    # h_{-1} = 0: h_0 = relu(xw_0)
                    nc.vector.tensor_scalar_max(
                        r3_h(hc[0])[:, :, 0:BL],
                        xwc[0][:, 0:W].rearrange("p (q b) -> p q b", b=BL),
                        0.0)
                    if K_COMPACT:
                        nc.vector.tensor_scalar_max(
                            hcur[z_tag][0][:], xwc[0][:, 0:W], 0.0)
                    return
                pc, pr = divmod(t - 1, CH)
                ps = p_psr.tile([128, KH * BL], f32, tag=z_tag + "ps",
                                name=f"ps{z_tag}{t}")
                for k in range(KH):
                    if K_COMPACT:
                        rhs = hcur[z_tag][(t - 1) % NHB][:, k * BL:(k + 1) * BL]
                    else:
                        rhs = hc[pc][:, k * CH * BL + pr * BL:
                                     k * CH * BL + (pr + 1) * BL]
                    for m in range(KH):
                        nc.tensor.matmul(
                            ps[:, m * BL:(m + 1) * BL],
                            whh_sb[:, k * H + m * 128: k * H + (m + 1) * 128],
                            rhs,
                            start=(k == 0),
                            stop=(k == KH - 1),
                        )
                if K_EPI == 'none':
                    return
                z = p_z.tile([128, KH * BL], bf, tag=z_tag,
                             name=f"z{z_tag}{t}")
                nc.vector.tensor_tensor(
                    z[:], ps[:], xwc[c][:, r * W:(r + 1) * W], AluOpType.add)
                hdst = r3_h(hc[c])[:, :, r * BL:(r + 1) * BL]
                if K_EPI == 'detached':
                    # timing ablation: write h into a scratch tile so the
                    # PE's next-step reads never wait on this DVE op
                    zs = p_z.tile([128, KH * BL], bf, tag=z_tag + "scr",
                                  name=f"zs{z_tag}{t}")
                    hdst = zs[:].rearrange("p (q b) -> p q b", b=BL)
                if K_COMPACT:
                    # chain-critical: compact tile for the next step's matmuls
                    nc.vector.tensor_scalar_max(
                        hcur[z_tag][t % NHB][:], z[:], 0.0)
                nc.vector.tensor_scalar_max(
                    hdst,
                    z[:].rearrange("p (q b) -> p q b", b=BL),
                    0.0)

            # ---- final FC + sigmoid for a finished h1 chunk ----
            def fc(c):
                ps = p_psfc.tile([MF, CH * BL], f32, tag="psfc", name=f"psfc{c}")
                for k in range(KH):
                    nc.tensor.matmul(
                        ps[:],
                        wfc_sb[:, k * MF:(k + 1) * MF],
                        h1c[c][:, k * CH * BL:(k + 1) * CH * BL],
                        start=(k == 0),
                        stop=(k == KH - 1),
                    )
                o = p_fco.tile([1, CH * BL], f32, tag="fco", name=f"fco{c}")
                nc.scalar.activation(
                    o[:], ps[0:1, :], mybir.ActivationFunctionType.Sigmoid,
                    bias=bfc_sb[0:1, 0:1],
                )
                nc.sync.dma_start(y_d[c:c + 1, :], o[:])

            # ---- interleaved schedule ----
            # gemm0 chunk 0 upfront; later chunks stream one ahead of the
            # L0 recurrence inside the loop so the PE head stays short.
            K_NO_GEMM0 = bool(os.environ.get('K_NO_GEMM0'))
            if not K_NO_GEMM0:
                gemm0(0)
                if nsc > 1:
                    gemm0(1)
            for t in range(n_steps + delay):
                if (not K_NO_GEMM0 and t < n_steps and t % CH == 0
                        and t // CH + 2 < nsc):
                    gemm0(t // CH + 2)
                if t < n_steps:
                    rec_step(t, w0h_sb, xw0c, h0c, "z0")
                if t >= delay:
                    _xw1 = xw0c if os.environ.get("K_NO_GEMM1") else xw1c
                    rec_step(t - delay, w1h_sb, _xw1, h1c, "z1")
                if t < n_steps and (t + 1) % CH == 0:
                    if not os.environ.get("K_NO_GEMM1"):
                        gemm1((t + 1) // CH - 1)
                if t >= delay and (t - delay + 1) % CH == 0:
                    if not os.environ.get("K_NO_FC"):
                        fc((t - delay + 1) // CH - 1)
            if os.environ.get("K_NO_FC"):
                # still need the y output written once
                o0 = p_fco.tile([1, CH * BL], f32, tag="fco", name="fco_x")
                nc.vector.memset(o0[:], 0.0)
                for c in range(nsc):
                    nc.sync.dma_start(y_d[c:c + 1, :], o0[:])
            if debug_dumps:
                dc = int(os.environ.get("K_DBG_CHUNK", "0"))
                nc.sync.dma_start(dbg["dxw0"][:], xw0c[dc][:])
                nc.sync.dma_start(dbg["dh0"][:], h0c[dc][:])
                nc.sync.dma_start(dbg["dxw1"][:], xw1c[dc][:])
                nc.sync.dma_start(dbg["dh1"][:], h1c[dc][:])

    if split_waits:
        _split_multi_waits(nc)
    return nc


_cache = {}


def _get_nc(n_steps):
    if n_steps not in _cache:
        _cache[n_steps] = build_nc(n_steps)
    return _cache[n_steps]


def _wfc_host(W_fc):
    MF = 8
    w = np.zeros((KH, 128, MF), np.float32)
    w[:, :, 0] = W_fc.reshape(KH, 128)
    return np.ascontiguousarray(w.transpose(1, 0, 2).reshape(128, KH * MF)).astype(BF16)


def _prep_inputs(x, W_ih0, W_hh0, b_ih0, b_hh0, W_ih1, W_hh1, b_ih1, b_hh1,
                 W_fc, b_fc, n_steps=T):
    shared = {
        "w0i": np.ascontiguousarray(W_ih0.T).astype(BF16),
        "w0h": np.ascontiguousarray(W_hh0.T).astype(BF16),
        "w1i": np.ascontiguousarray(W_ih1.T).astype(BF16),
        "w1h": np.ascontiguousarray(W_hh1.T).astype(BF16),
        "wfc": _wfc_host(W_fc),
        "b0": np.ascontiguousarray((b_ih0 + b_hh0).reshape(KH, 128).T).astype(
            np.float32),
        "b1": np.ascontiguousarray((b_ih1 + b_hh1).reshape(KH, 128).T).astype(
            np.float32),
        "bfc": b_fc.reshape(1, 1).astype(np.float32),
    }
    in_maps = []
    for c in range(NCORES):
        xs = x[c * BL:(c + 1) * BL, :n_steps]          # [BL, n_steps, I]
        xt = np.ascontiguousarray(xs.transpose(2, 1, 0)).reshape(
            I, n_steps * BL)                            # col = t*BL + b
        in_maps.append({"xt": xt.astype(BF16), **shared})
    return in_maps


def _postprocess(results, n_steps=T):
    outs = []
    for c in range(NCORES):
        y = results[c]["y"].reshape(n_steps, BL)        # [t, b]
        outs.append(y.T)                                # [b, t]
    return np.concatenate(outs, axis=0)[:, :, None].astype(np.float32)


def kernel(x, W_ih0, W_hh0, b_ih0, b_hh0, W_ih1, W_hh1, b_ih1, b_hh1,
           W_fc, b_fc):
    x, W_ih0, W_hh0, b_ih0, b_hh0, W_ih1, W_hh1, b_ih1, b_hh1, W_fc, b_fc = [
        np.asarray(a, dtype=np.float32)
        for a in (x, W_ih0, W_hh0, b_ih0, b_hh0, W_ih1, W_hh1, b_ih1, b_hh1,
                  W_fc, b_fc)
    ]
    nc = _get_nc(T)
    in_maps = _prep_inputs(x, W_ih0, W_hh0, b_ih0, b_hh0, W_ih1, W_hh1,
                           b_ih1, b_hh1, W_fc, b_fc)
    res = run_bass_kernel_spmd(nc, in_maps, core_ids=list(range(NCORES)))
    return _postprocess(res.results)

